# revision 29
# baseline (speedup 1.0000x reference)
"""Trainium2 Bass kernel for the MAMBA-flow model (B=16, NS=NQ=1024, D=256,
L=4 layers, S=8 state dims, 4 attention heads).

Sharding: data-parallel over batch, 2 batch elements per core on 8 cores.

Layouts per core (BL=2 batch elements, N=2048 tokens each, T=4096 tokens):
 - seq "a"-layout: (128 tokens, 256 d) tiles, 32 of them (token-major).
 - xnT "b"-layout: (128 d, 4096 tokens) x 2 d-blocks (feature-major).
 - SSM scan state: (16 rows = batch*s, 2048 time) on the vector engine via
   tensor_tensor_scan (state = A*state + Bu along the free axis).
All weight folds (ln_g into gate/B/D weights, 1/sqrt(hd) into Wq, transposes
into lhsT layouts, the A_bar table) are host-side numpy on the actual inputs.
"""
import os
import sys

import numpy as np

try:
    import concourse.bass as bass  # noqa: F401
except Exception:  # pragma: no cover
    sys.path.insert(0, "/opt/trn_rl_repo")

import concourse.bass as bass
import concourse.mybir as mybir
from concourse import bacc
from concourse.tile import TileContext
from concourse.masks import make_identity

F32 = mybir.dt.float32
AF = mybir.ActivationFunctionType
OP = mybir.AluOpType

N_CORES = 8
B, NS, NQ = 16, 1024, 1024
D, L, S, H, HD = 256, 4, 8, 4, 64
N = NS + NQ            # tokens per batch element
BL = B // N_CORES      # batch elements per core
T = BL * N             # tokens per core
NT = T // 128          # token tiles per core (32)
LN_EPS = 1e-5
PI_HALF = float(np.pi / 2.0)

_cache = {}


def _assert_zero(x, name):
    if np.abs(np.asarray(x)).max() != 0.0:
        raise NotImplementedError(f"kernel assumes {name} == 0")


def _prep_consts(inp):
    """Host-side weight folding. Returns dict name -> np.ndarray (shared)."""
    f32 = lambda a: np.ascontiguousarray(np.asarray(a, np.float32))
    c = {}
    bf = f32(inp["B_f"])                                   # (2, 16)
    c["bf"] = np.concatenate([bf, bf], axis=1)             # (2, 32) sin|cos
    sc_bias = np.zeros((32, 1), np.float32)
    sc_bias[16:] = np.float32(np.pi / 2.0)
    c["sincos_bias"] = sc_bias
    featconst = np.zeros((3, T), np.float32)
    featconst[0] = 1.0                                     # ones (bias feat)
    for _b in range(BL):
        featconst[1 + _b, _b * N:(_b + 1) * N] = 1.0       # batch indicator
    c["featconst"] = featconst
    # input projections, lhsT layout (feat 33 -> row 33 gets bias + t_emb)
    win = np.zeros((2, 34, D), np.float32)
    win[0, :33] = f32(inp["Wi_in"]).T
    win[1, :33] = f32(inp["Wq_in"]).T
    win[0, 33] = f32(inp["bi_in"])
    win[1, 33] = f32(inp["bq_in"])
    c["win"] = win
    half = D // 2
    freqs = np.exp(np.arange(half, dtype=np.float32) *
                   (-np.log(10000.0) / (half - 1))).astype(np.float32)
    c["freqs2"] = np.concatenate([freqs, freqs])[None, :]  # (1, 256)

    ln_g = f32(inp["ln_g"])
    _assert_zero(inp["ln_b"], "ln_b")
    _assert_zero(inp["gate_b"], "gate_b")
    gate_w = f32(inp["gate_w"])                            # (L, D, D)
    Bw = f32(inp["Bw"])                                    # (L, S, D)
    Cw = f32(inp["Cw"])                                    # (L, D, S)
    Dp = f32(inp["Dp"])                                    # (L, D)
    A_log = f32(inp["A_log"])                              # (L, S)

    wg = np.zeros((L, 2, 128, D), np.float32)
    bwT = np.zeros((L, 2, 128, S), np.float32)
    cwT = np.zeros((L, 40, D), np.float32)
    dg = np.zeros((L, 2, 128, D), np.float32)
    a_rep = np.zeros((L, 40, N), np.float32)
    for l in range(L):
        G = (gate_w[l] * ln_g[l][None, :]).T               # (d_in, d_out)
        wg[l] = G.reshape(2, 128, D)
        Bf = (Bw[l] * ln_g[l][None, :]).T                  # (d_in=256, S)
        bwT[l] = Bf.reshape(2, 128, S)
        cwT[l, 0:S] = Cw[l].T
        cwT[l, 32:32 + S] = Cw[l].T
        dvals = Dp[l] * ln_g[l]
        dg[l, 0, :, 0:128] = np.diag(dvals[:128])
        dg[l, 1, :, 128:256] = np.diag(dvals[128:])
        A = -np.clip(np.exp(A_log[l]), 1e-8, 10.0).astype(np.float32)
        A_bar = np.exp(A / np.float32(N)).astype(np.float32)   # (S,)
        a_rep[l, 0:S] = np.repeat(A_bar[:, None], N, axis=1)
        a_rep[l, 32:32 + S] = np.repeat(A_bar[:, None], N, axis=1)
    c["wg"], c["bwT"], c["cwT"], c["dg"], c["a_rep"] = wg, bwT, cwT, dg, a_rep

    ipw = f32(inp["in_proj_w"])                            # (3D, D)
    _assert_zero(inp["in_proj_b"], "in_proj_b")
    _assert_zero(inp["out_b"], "out_b")
    _assert_zero(inp["dec_b1"], "dec_b1")
    c["wqT"] = (ipw[:D] * np.float32(1.0 / np.sqrt(HD))).T.reshape(2, 128, D)
    c["wkT"] = ipw[D:2 * D].T.reshape(2, 128, D).copy()
    c["wvT"] = ipw[2 * D:].T.reshape(2, 128, D).copy()
    c["woT"] = f32(inp["out_w"]).T.reshape(2, 128, D).copy()
    c["w1T"] = f32(inp["dec_w1"]).T.reshape(2, 128, D).copy()
    c["w2T"] = f32(inp["dec_w2"]).T.reshape(2, 128, 1).copy()
    c = {k: np.ascontiguousarray(v) for k, v in c.items()}
    return c


def _prep_core_inputs(inp, ci):
    """Per-core marshaled inputs (host-side reshape/transpose only)."""
    f32 = lambda a: np.asarray(a, np.float32)
    sc = f32(inp["sparse_coords"])   # (B, NS, 2)
    qc = f32(inp["query_coords"])    # (B, NQ, 2)
    sv = f32(inp["sparse_values"])   # (B, NS, 1)
    nz = f32(inp["noise"])           # (B, NQ, 1)
    t = f32(inp["t"])                # (B,)
    coordsT = np.zeros((2, T), np.float32)
    valnoise = np.zeros((1, T), np.float32)
    for b in range(BL):
        g = BL * ci + b
        coordsT[:, b * N: b * N + NS] = sc[g].T
        coordsT[:, b * N + NS: (b + 1) * N] = qc[g].T
        valnoise[0, b * N: b * N + NS] = sv[g, :, 0]
        valnoise[0, b * N + NS: (b + 1) * N] = nz[g, :, 0]
    t_row = t[BL * ci: BL * ci + BL][None, :].copy()       # (1, BL)
    return {"coordsT": coordsT, "valnoise": valnoise, "t_row": t_row}


def build(consts, iters=0, debug=False):
    """Build the Bass program. iters=0 -> straight-line; iters=k -> wrap the
    whole body in a hardware For_i loop with k iterations (for timing)."""
    nc = bacc.Bacc()

    d_in = {}
    d_in["coordsT"] = nc.dram_tensor("coordsT", [2, T], F32, kind="ExternalInput")
    d_in["valnoise"] = nc.dram_tensor("valnoise", [1, T], F32, kind="ExternalInput")
    d_in["t_row"] = nc.dram_tensor("t_row", [1, BL], F32, kind="ExternalInput")
    for k, v in consts.items():
        d_in[k] = nc.dram_tensor(k, list(v.shape), F32, kind="ExternalInput")
    y_out = nc.dram_tensor("y", [BL, NQ], F32, kind="ExternalOutput")
    dbg = {}
    if debug:
        dbg["featsT"] = nc.dram_tensor("dbg_featsT", [36, T], F32, kind="ExternalOutput")
        dbg["win_aug"] = nc.dram_tensor("dbg_win_aug", [36, 2, D], F32, kind="ExternalOutput")
        for l in range(L + 1):
            dbg[f"seq{l}"] = nc.dram_tensor(f"dbg_seq{l}", [128, NT, D], F32, kind="ExternalOutput")
        dbg["xnT0"] = nc.dram_tensor("dbg_xnT0", [128, 2, T], F32, kind="ExternalOutput")
        dbg["xnT0e"] = nc.dram_tensor("dbg_xnT0e", [128, 2, T], F32, kind="ExternalOutput")
        dbg["xnp0"] = nc.dram_tensor("dbg_xnp0", [128, NT, D], F32, kind="ExternalOutput")
        dbg["gate0"] = nc.dram_tensor("dbg_gate0", [128, NT, D], F32, kind="ExternalOutput")
        dbg["gpre0"] = nc.dram_tensor("dbg_gpre0", [128, NT, D], F32, kind="ExternalOutput")
        dbg["wg_sb"] = nc.dram_tensor("dbg_wg_sb", [128, L, 2, D], F32, kind="ExternalOutput")
        dbg["bu0"] = nc.dram_tensor("dbg_bu0", [40, N], F32, kind="ExternalOutput")
        dbg["hs0"] = nc.dram_tensor("dbg_hs0", [40, N], F32, kind="ExternalOutput")
        dbg["qT"] = nc.dram_tensor("dbg_qT", [128, 2, BL * NQ], F32, kind="ExternalOutput")
        dbg["kT"] = nc.dram_tensor("dbg_kT", [128, 2, BL * NS], F32, kind="ExternalOutput")
        dbg["o_normT"] = nc.dram_tensor("dbg_o_normT", [128, 2, BL * NQ], F32, kind="ExternalOutput")

    with TileContext(nc) as tc:
        _build_body(nc, tc, d_in, y_out, iters, dbg)
    nc.compile()
    return nc


def _build_body(nc, tc, d_in, y_out, iters, dbg=None):
    from contextlib import ExitStack
    with ExitStack() as ctx:
        constp = ctx.enter_context(tc.tile_pool(name="const", bufs=1))
        # ---- load constants into SBUF ----
        cs = {}
        cs["bf"] = constp.tile([2, 32], F32, name="bf", tag="bf")
        cs["win"] = constp.tile([34, 2, D], F32, name="win", tag="win")
        cs["freqs2"] = constp.tile([1, D], F32, name="freqs2", tag="freqs2")
        cs["wg"] = constp.tile([128, L, 2, D], F32, name="wg", tag="wg")
        cs["bwT"] = constp.tile([128, L, 2, S], F32, name="bwT", tag="bwT")
        cs["cwT"] = constp.tile([40, L, D], F32, name="cwT", tag="cwT")
        cs["dg"] = constp.tile([128, L, 2, D], F32, name="dg", tag="dg")
        cs["wqT"] = constp.tile([128, 2, D], F32, name="wqT", tag="wqT")
        cs["wkT"] = constp.tile([128, 2, D], F32, name="wkT", tag="wkT")
        cs["wvT"] = constp.tile([128, 2, D], F32, name="wvT", tag="wvT")
        cs["woT"] = constp.tile([128, 2, D], F32, name="woT", tag="woT")
        cs["w1T"] = constp.tile([128, 2, D], F32, name="w1T", tag="w1T")
        cs["w2T"] = constp.tile([128, 2, 1], F32, name="w2T", tag="w2T")
        nc.sync.dma_start(out=cs["bf"], in_=d_in["bf"][:, :])
        nc.sync.dma_start(out=cs["freqs2"], in_=d_in["freqs2"][:, :])
        for seg in range(2):
            nc.sync.dma_start(out=cs["win"][:, seg, :], in_=d_in["win"][seg])
        for l in range(L):
            for kb in range(2):
                nc.sync.dma_start(out=cs["wg"][:, l, kb, :], in_=d_in["wg"][l, kb])
                nc.sync.dma_start(out=cs["bwT"][:, l, kb, :], in_=d_in["bwT"][l, kb])
                nc.sync.dma_start(out=cs["dg"][:, l, kb, :], in_=d_in["dg"][l, kb])
            nc.sync.dma_start(out=cs["cwT"][:, l, :], in_=d_in["cwT"][l])
        for w in ("wqT", "wkT", "wvT", "woT", "w1T", "w2T"):
            for kb in range(2):
                nc.sync.dma_start(out=cs[w][:, kb, :], in_=d_in[w][kb])
        ident = constp.tile([128, 128], F32, name="ident")
        make_identity(nc, ident)
        eps_col = constp.tile([128, 1], F32, name="eps_col")
        nc.vector.memset(eps_col, LN_EPS)
        ones_row = constp.tile([1, 64], F32, name="ones_row")
        nc.vector.memset(ones_row, 1.0)
        pih_col = constp.tile([128, 1], F32, name="pih_col")
        nc.vector.memset(pih_col, PI_HALF)

        def body(_=None):
            _model(nc, tc, d_in, y_out, cs, ident, eps_col, ones_row, pih_col, dbg or {})

        if iters == 0:
            body()
        else:
            with tc.For_i(0, iters, 1):
                body()


def _model(nc, tc, d_in, y_out, cs, ident, eps_col, ones_row, pih_col, dbg):
    from contextlib import ExitStack
    ctx = ExitStack()
    ctx.__enter__()
    ps = ctx.enter_context(tc.tile_pool(name="ps", bufs=8, space="PSUM"))
    bigp = ctx.enter_context(tc.tile_pool(name="big", bufs=1))

    # persistent activations
    seq = bigp.tile([128, NT, D], F32, name="seq", tag="seq")          # token-major
    xnT = bigp.tile([128, 2, T], F32, name="xnT", tag="xnT")           # feature-major
    gate = bigp.tile([128, NT, D], F32, name="gate", tag="gate")

    # ---------------- embed ----------------
    # featsT rows: 0-15 sin, 16-31 cos (one 32-row Sin with per-partition
    # bias), 32 val/noise, 33 ones (bias feature), 34/35 batch indicators
    # (select which batch's t_emb row applies). Rows 32-35 land via DMA.
    with tc.tile_pool(name="embed", bufs=1) as ep, \
         tc.tile_pool(name="embed_s", bufs=1) as esp:
        coordsT = ep.tile([2, T], F32)
        nc.sync.dma_start(out=coordsT, in_=d_in["coordsT"][:, :])
        t_sb = ep.tile([1, BL], F32)
        nc.sync.dma_start(out=t_sb, in_=d_in["t_row"][:, :])
        sc_bias = ep.tile([32, 1], F32)
        nc.sync.dma_start(out=sc_bias, in_=d_in["sincos_bias"][:, :])
        featsT = ep.tile([36, T], F32)
        nc.sync.dma_start(out=featsT[32:33, :], in_=d_in["valnoise"][:, :])
        nc.sync.dma_start(out=featsT[33:36, :], in_=d_in["featconst"][:, :])

        # fourier features: rows 16-31 get +pi/2 (cos) BEFORE the range
        # reduction x -> x - 2pi*round(x/2pi), keeping Sin args in [-pi, pi]
        TWO_PI = float(2.0 * np.pi)
        for c in range(T // 512):
            psp = ps.tile([32, 512], F32, name="ps", tag="ps")
            nc.tensor.matmul(psp, cs["bf"], coordsT[:, c * 512:(c + 1) * 512],
                             start=True, stop=True)
            xb = esp.tile([32, 512], F32, name="xb", tag="xb", bufs=2)
            nc.vector.tensor_scalar(xb, psp, sc_bias, None, op0=OP.add)
            MAGIC = float(1.5 * 2 ** 23)  # fp32 add forces round-to-nearest int
            kf = esp.tile([32, 512], F32, name="kf", tag="kf", bufs=2)
            nc.vector.tensor_scalar(kf, xb, 1.0 / TWO_PI, MAGIC,
                                    op0=OP.mult, op1=OP.add)
            xr = esp.tile([32, 512], F32, name="xr", tag="xr", bufs=2)
            nc.vector.tensor_scalar(xr, kf, MAGIC, -TWO_PI,
                                    op0=OP.subtract, op1=OP.mult)
            nc.vector.tensor_add(xr, xr, xb)
            nc.scalar.activation(featsT[0:32, c * 512:(c + 1) * 512], xr, AF.Sin)

        # time embedding -> rows 34/35 of the augmented weight (via DMA)
        win_aug = esp.tile([36, 2, D], F32)
        for seg in range(2):
            nc.sync.dma_start(out=win_aug[0:34, seg, :], in_=d_in["win"][seg])
        temb = esp.tile([128, 2, BL], F32)
        for half in range(2):
            pst = ps.tile([128, BL], F32, name="ps", tag="ps")
            nc.tensor.matmul(pst, cs["freqs2"][:, half * 128:(half + 1) * 128],
                             t_sb, start=True, stop=True)
            if half == 0:
                nc.scalar.activation(temb[:, half, :], pst, AF.Sin)
            else:
                nc.scalar.activation(temb[:, half, :], pst, AF.Sin, bias=pih_col)
        tembT = esp.tile([BL, D], F32)
        for half in range(2):
            pst2 = ps.tile([BL, 128], F32, name="ps", tag="ps")
            nc.tensor.transpose(pst2, temb[:, half, :], ident)
            nc.scalar.copy(tembT[:, half * 128:(half + 1) * 128], pst2)
        for seg in range(2):
            nc.sync.dma_start(out=win_aug[34:36, seg, :], in_=tembT)

        # input projection into seq (token-major)
        for i in range(NT):
            seg = (i // 8) % 2
            pse = ps.tile([128, D], F32, name="ps", tag="ps")
            nc.tensor.matmul(pse, featsT[:, i * 128:(i + 1) * 128],
                             win_aug[:, seg, :], start=True, stop=True)
            nc.scalar.copy(seq[:, i, :], pse)
        if "featsT" in dbg:
            nc.sync.dma_start(out=dbg["featsT"][:, :], in_=featsT)
            nc.sync.dma_start(out=dbg["win_aug"][:, :, :], in_=win_aug)
            nc.sync.dma_start(out=dbg["seq0"][:, :, :], in_=seq)

    # ---------------- mamba layers ----------------
    with tc.tile_pool(name="lay", bufs=1) as lp, \
         tc.tile_pool(name="lay_s", bufs=6) as sp, \
         tc.tile_pool(name="lay_m", bufs=4) as mp:
        gate = lp.tile([128, NT, D], F32, name="gate", tag="gate")
        bu = lp.tile([40, N], F32)
        hs = lp.tile([40, N], F32)
        nc.vector.memset(bu, 0.0)
        for l in range(L):
            a_rep_l = lp.tile([40, N], F32, name="a_rep_l", tag="a_rep_l", bufs=1)
            nc.sync.dma_start(out=a_rep_l, in_=d_in["a_rep"][l])
            # layernorm stats + apply + transpose
            for i in range(NT):
                stats = sp.tile([128, 6], F32, name="stats", tag="stats")
                nc.vector.bn_stats(stats, seq[:, i, :])
                mv = sp.tile([128, 2], F32, name="mv", tag="mv")
                nc.vector.bn_aggr(mv, stats)
                rstd = sp.tile([128, 1], F32, name="rstd", tag="rstd")
                nc.scalar.activation(rstd, mv[:, 1:2], AF.Sqrt, bias=eps_col)
                nc.vector.reciprocal(rstd, rstd)
                nmr = sp.tile([128, 1], F32, name="nmr", tag="nmr")
                nc.vector.tensor_scalar(nmr, mv[:, 0:1], rstd, -1.0,
                                        op0=OP.mult, op1=OP.mult)
                xnp = mp.tile([128, D], F32, name="xnp", tag="xnp")
                nc.vector.tensor_scalar(xnp, seq[:, i, :], rstd, nmr,
                                        op0=OP.mult, op1=OP.add)
                for hb in range(2):
                    pst = ps.tile([128, 128], F32, name="ps", tag="ps")
                    nc.tensor.transpose(pst, xnp[:, hb * 128:(hb + 1) * 128], ident)
                    nc.scalar.copy(xnT[:, hb, i * 128:(i + 1) * 128], pst)
                if l == 0 and "xnp0" in dbg:
                    nc.sync.dma_start(out=dbg["xnp0"][:, i, :], in_=xnp)
            if l == 0 and "xnT0e" in dbg:
                nc.sync.dma_start(out=dbg["xnT0e"][:, :, :], in_=xnT)

            if l == 0 and "wg_sb" in dbg:
                nc.sync.dma_start(out=dbg["wg_sb"][:, :, :, :], in_=cs["wg"])
            # gate = sigmoid(xn @ Wg'^T)  (token-major)
            for i in range(NT):
                psg = ps.tile([128, D], F32, name="ps", tag="ps")
                for kb in range(2):
                    nc.tensor.matmul(psg, xnT[:, kb, i * 128:(i + 1) * 128],
                                     cs["wg"][:, l, kb, :],
                                     start=(kb == 0), stop=(kb == 1))
                nc.scalar.activation(gate[:, i, :], psg, AF.Sigmoid)
                if l == 0 and "gpre0" in dbg:
                    gpre = mp.tile([128, D], F32, name="gpre", tag="gpre")
                    nc.vector.tensor_copy(gpre, psg)
                    nc.sync.dma_start(out=dbg["gpre0"][:, i, :], in_=gpre)

            # Bu = Bw' @ xnT  -> (40, N) rows {0-7, 32-39}, then scan
            for c in range(N // 512):
                psb = ps.tile([40, 512], F32, name="ps", tag="ps")
                for b in range(BL):
                    po = 32 * b
                    for kb in range(2):
                        nc.tensor.matmul(
                            psb[po:po + S, :],
                            cs["bwT"][:, l, kb, :],
                            xnT[:, kb, b * N + c * 512: b * N + (c + 1) * 512],
                            start=(kb == 0), stop=(kb == 1))
                    nc.scalar.copy(bu[po:po + S, c * 512:(c + 1) * 512],
                                   psb[po:po + S, :])
            nc.vector.tensor_tensor_scan(hs, a_rep_l, bu,
                                         0.0, OP.mult, OP.add)

            # ys = Dp*xn + Cw' @ hs ; seq += gate * ys
            for i in range(NT):
                b, cch = i // 16, i % 16
                psy = ps.tile([128, D], F32, name="ps", tag="ps")
                nc.tensor.matmul(psy,
                                 hs[32 * b:32 * b + S,
                                    cch * 128:(cch + 1) * 128],
                                 cs["cwT"][32 * b:32 * b + S, l, :],
                                 start=True, stop=False)
                for hb in range(2):
                    nc.tensor.matmul(psy,
                                     xnT[:, hb, i * 128:(i + 1) * 128],
                                     cs["dg"][:, l, hb, :],
                                     start=False, stop=(hb == 1))
                tmp = mp.tile([128, D], F32, name="tmp", tag="tmp")
                nc.vector.tensor_mul(tmp, gate[:, i, :], psy)
                nc.vector.tensor_add(seq[:, i, :], seq[:, i, :], tmp)

    # ---------------- cross attention + decoder ----------------
    with tc.tile_pool(name="attn", bufs=1) as ap, \
         tc.tile_pool(name="attn_s", bufs=4) as asp:
        seqT = xnT  # reuse the slot: same shape, layers done with it
        for i in range(NT):
            for hb in range(2):
                pst = ps.tile([128, 128], F32, name="ps", tag="ps")
                nc.tensor.transpose(pst, seq[:, i, hb * 128:(hb + 1) * 128], ident)
                nc.scalar.copy(seqT[:, hb, i * 128:(i + 1) * 128], pst)

        qT = ap.tile([128, 2, BL * NQ], F32, name="qT", tag="qT")
        kT = ap.tile([128, 2, BL * NS], F32, name="kT", tag="kT")
        for b in range(BL):
            for m in range(2):
                for c in range(NQ // 512):
                    dst = b * NQ + c * 512
                    psq = ps.tile([128, 512], F32, name="ps", tag="ps")
                    for kb in range(2):
                        nc.tensor.matmul(
                            psq, cs["wqT"][:, kb, m * 128:(m + 1) * 128],
                            seqT[:, kb, b * N + NS + c * 512: b * N + NS + (c + 1) * 512],
                            start=(kb == 0), stop=(kb == 1))
                    nc.scalar.copy(qT[:, m, dst:dst + 512], psq)
                    psk = ps.tile([128, 512], F32, name="ps", tag="ps")
                    for kb in range(2):
                        nc.tensor.matmul(
                            psk, cs["wkT"][:, kb, m * 128:(m + 1) * 128],
                            seqT[:, kb, b * N + c * 512: b * N + (c + 1) * 512],
                            start=(kb == 0), stop=(kb == 1))
                    nc.scalar.copy(kT[:, m, dst:dst + 512], psk)

        # v in token-major layout, per head + ones column for sum(exp)
        vsb = ap.tile([128, BL * 8, H, HD + 1], F32, name="vsb", tag="vsb")
        for idx in range(BL * 8):
            b, ti = idx // 8, idx % 8
            psv = ps.tile([128, D], F32, name="ps", tag="ps")
            for kb in range(2):
                nc.tensor.matmul(psv,
                                 seqT[:, kb, b * N + ti * 128: b * N + (ti + 1) * 128],
                                 cs["wvT"][:, kb, :], start=(kb == 0), stop=(kb == 1))
            for h in range(H):
                nc.scalar.copy(vsb[:, idx, h, 0:HD], psv[:, h * HD:(h + 1) * HD])
            nc.vector.memset(vsb[:, idx, :, HD:HD + 1], 1.0)

        o_normT = ap.tile([128, 2, BL * NQ], F32, name="o_normT", tag="o_normT")
        # center keys per (batch, dim): softmax(q . (k - kmean)) is identical
        # (per-query constant shift) but keeps exp() in fp32 range
        for m in range(2):
            for b in range(BL):
                ksum = asp.tile([128, 1], F32, name="ksum", tag="ksum", bufs=2)
                nc.vector.tensor_reduce(ksum, kT[:, m, b * NS:(b + 1) * NS],
                                        axis=mybir.AxisListType.X, op=OP.add)
                nc.vector.tensor_scalar_mul(ksum, ksum, 1.0 / NS)
                nc.vector.tensor_scalar(kT[:, m, b * NS:(b + 1) * NS],
                                        kT[:, m, b * NS:(b + 1) * NS],
                                        ksum, None, op0=OP.subtract)

        for b in range(BL):
            for h in range(H):
                po, mo = 64 * (h % 2), h // 2
                for c2 in range(NQ // 512):
                    opsum = ps.tile([HD + 1, 512], F32, name="ps", tag="ps")
                    for t8 in range(8):
                        pssc = ps.tile([128, 512], F32, name="ps", tag="ps")
                        nc.tensor.matmul(
                            pssc,
                            kT[po:po + 64, mo, b * NS + t8 * 128: b * NS + (t8 + 1) * 128],
                            qT[po:po + 64, mo, b * NQ + c2 * 512: b * NQ + (c2 + 1) * 512],
                            start=True, stop=True)
                        expt = asp.tile([128, 512], F32, name="expt", tag="expt", bufs=2)
                        nc.scalar.activation(expt, pssc, AF.Exp)
                        nc.tensor.matmul(opsum, vsb[:, b * 8 + t8, h, :], expt,
                                         start=(t8 == 0), stop=(t8 == 7))
                    rec = asp.tile([1, 512], F32, name="rec", tag="rec", bufs=1)
                    nc.vector.reciprocal(rec, opsum[HD:HD + 1, :])
                    bc = ps.tile([64, 512], F32, name="ps", tag="ps")
                    nc.tensor.matmul(bc, ones_row, rec, start=True, stop=True)
                    ou = asp.tile([64, 512], F32, name="ou", tag="ou", bufs=2)
                    nc.scalar.copy(ou, opsum[0:HD, :])
                    nc.vector.tensor_mul(
                        o_normT[po:po + 64, mo, b * NQ + c2 * 512: b * NQ + (c2 + 1) * 512],
                        ou, bc)

        if "qT" in dbg:
            nc.sync.dma_start(out=dbg["qT"][:, :, :], in_=qT)
            nc.sync.dma_start(out=dbg["kT"][:, :, :], in_=kT)
            nc.sync.dma_start(out=dbg["o_normT"][:, :, :], in_=o_normT)
        attT = ap.tile([128, 2, BL * NQ], F32, name="attT", tag="qT")
        for m in range(2):
            for c in range(BL * NQ // 512):
                psa = ps.tile([128, 512], F32, name="ps", tag="ps")
                for kb in range(2):
                    nc.tensor.matmul(psa, cs["woT"][:, kb, m * 128:(m + 1) * 128],
                                     o_normT[:, kb, c * 512:(c + 1) * 512],
                                     start=(kb == 0), stop=(kb == 1))
                nc.scalar.copy(attT[:, m, c * 512:(c + 1) * 512], psa)
        h1T = ap.tile([128, 2, BL * NQ], F32, name="h1T", tag="kT")
        for m in range(2):
            for c in range(BL * NQ // 512):
                psd = ps.tile([128, 512], F32, name="ps", tag="ps")
                for kb in range(2):
                    nc.tensor.matmul(psd, cs["w1T"][:, kb, m * 128:(m + 1) * 128],
                                     attT[:, kb, c * 512:(c + 1) * 512],
                                     start=(kb == 0), stop=(kb == 1))
                nc.scalar.activation(h1T[:, m, c * 512:(c + 1) * 512], psd, AF.Relu)
        for c in range(BL * NQ // 512):
            pso = ps.tile([1, 512], F32, name="ps", tag="ps")
            for kb in range(2):
                nc.tensor.matmul(pso, cs["w2T"][:, kb, :],
                                 h1T[:, kb, c * 512:(c + 1) * 512],
                                 start=(kb == 0), stop=(kb == 1))
            orow = asp.tile([1, 512], F32, name="orow", tag="orow", bufs=1)
            nc.scalar.copy(orow, pso)
            b = (c * 512) // NQ
            off = (c * 512) % NQ
            nc.sync.dma_start(out=y_out[b:b + 1, off:off + 512], in_=orow)
    ctx.close()


def _get_built(inp, iters=0):
    key = ("nc", iters)
    if key not in _cache:
        consts = _prep_consts(inp)
        _cache[key] = (build(consts, iters=iters), consts)
    return _cache[key]


def kernel(**inputs) -> np.ndarray:
    from concourse.bass_utils import run_bass_kernel_spmd
    nc, consts = _get_built(inputs, iters=0)
    in_maps = []
    for ci in range(N_CORES):
        m = dict(consts)
        m.update(_prep_core_inputs(inputs, ci))
        in_maps.append(m)
    res = run_bass_kernel_spmd(nc, in_maps, list(range(N_CORES)))
    out = np.zeros((B, NQ, 1), np.float32)
    dec_b2 = float(np.asarray(inputs["dec_b2"]).reshape(-1)[0])
    for ci in range(N_CORES):
        y = res.results[ci]["y"]  # (BL, NQ)
        out[BL * ci: BL * ci + BL, :, 0] = y + dec_b2
    return out
